# revision 11
# baseline (speedup 1.0000x reference)
"""Trainium2 Bass kernel for nn_EquivariantDecoder.

Data-parallel over 8 NeuronCores (batch sharded, 2048 rows/core).

The execution environment charges a large fixed cost per *unique* NEFF
instruction, while For_i loops re-execute instructions at ~2us per
back-edge.  The kernel is therefore a minimal program (~80
instructions) of dynamic loops:

  - host pre-transposes v into 38 matmul-ready [128, B] slots (bf16)
    and packs all layer weights (block-diagonal per-irrep) into one
    [128, NWC] bf16 tensor;
  - per b-tile of 512 rows: one DMA stages the tile's 38 slots into
    SBUF (the only t-indexed access; inner-loop access patterns are
    m-indexed only, which keeps the symbolic-AP register budget small);
  - each layer's per-m matmuls run under For_i with shared
    (weight-stationary) block-diagonal weights; gates applied in-place
    by DVE so layers 2/3 reuse the same h buffers;
  - layer 4 accumulates all 49 outputs into one PSUM bank; the host
    de-transposes the [49, 2048] per-core result.
"""

import numpy as np
import ml_dtypes
from contextlib import ExitStack

import concourse.bass as bass
import concourse.mybir as mybir
import concourse.tile as tile
from concourse.bass import ds, ts
from concourse import bass_utils

BF16 = mybir.dt.bfloat16
FP32 = mybir.dt.float32
bf = ml_dtypes.bfloat16

# ---------------- problem constants (hardcoded) ----------------
B_FULL = 16384
NCORES = 8
BC = B_FULL // NCORES          # 2048 rows per core
BT = 512                       # b-tile
NT = BC // BT                  # 4

IN_IRREPS = [(256, 0), (128, 1), (128, 2), (64, 3), (64, 4), (64, 5), (64, 6)]
HID_IRREPS = [(64, 0), (64, 1), (64, 2), (32, 3), (32, 4), (32, 5), (32, 6)]
N_SCALARS = 64
N_GATES = 256
D_IN = 3840
D_OUT = 49

IN_OFF = {}
_o = 0
for _mul, _l in IN_IRREPS:
    IN_OFF[_l] = _o
    _o += _mul * (2 * _l + 1)

# v slot map (38 slots of [128, B]): 2 l0-kslots | 5 l1 | 5 l2 | 13 B56 | 13 B34
S_L0 = 0
S_L1 = 2                       # l1 m-slots (m=3,4 zero)
S_L2 = 7
S_B56 = 12                     # rows 0:64 = l6 (all m), rows 64:128 = l5 (m<11)
S_B34 = 25                     # rows 0:64 = l4 (m<9),  rows 64:128 = l3 (m<7)
NSLOT = 38

_BUILD = {}
REPEAT = 1


def _pack_weights(w1, w2, w3, w4):
    """Pack all weights into one [128, NWC] bf16 matrix (columns).
    Returns (wt, col) where col maps name -> column offset."""

    def split_blocks(wflat, in_irr, out_irr):
        mul_in = {l: m for m, l in in_irr}
        blocks = []
        off = 0
        for mo, l in out_irr:
            mi = mul_in[l]
            w = wflat[off:off + mi * mo].reshape(mi, mo) / np.sqrt(mi)
            off += mi * mo
            blocks.append((l, w))
        assert off == wflat.size
        return blocks

    pre_irr = [(N_SCALARS, 0), (N_GATES, 0)] + [(m, l) for m, l in HID_IRREPS if l > 0]
    # gate channel order: gA = [g_l2 | g_l1], gB = [g_l6 | g_l5 | g_l4 | g_l3]
    gperm = ([64 + i for i in range(64)] + [i for i in range(64)] +
             [224 + i for i in range(32)] + [192 + i for i in range(32)] +
             [160 + i for i in range(32)] + [128 + i for i in range(32)])

    segs = []            # (name, [128, w] fp32 array)

    def add(name, arr):
        a = np.zeros((128, arr.shape[1]), np.float32)
        a[:arr.shape[0]] = arr
        segs.append((name, a))

    # ---- layer 1 ----
    b1 = split_blocks(w1, IN_IRREPS, pre_irr)
    ws, wg = b1[0][1], b1[1][1]                      # [256,64], [256,256]
    W10 = np.concatenate([ws, wg[:, gperm]], axis=1)  # [256, 320]
    add("W1_0a", W10[:128])
    add("W1_0b", W10[128:])
    w1l = {l: w for l, w in b1[2:]}
    add("W1_l1", w1l[1])                             # [128, 64] -> l1 out
    add("W1_l2", w1l[2])                             # [128, 64] -> l2 out
    wb56 = np.zeros((128, 64), np.float32)
    wb56[0:64, 0:32] = w1l[6]
    wb56[64:128, 32:64] = w1l[5]
    add("W1_b56", wb56)
    wb34 = np.zeros((128, 64), np.float32)
    wb34[0:64, 0:32] = w1l[4]
    wb34[64:128, 32:64] = w1l[3]
    add("W1_b34", wb34)

    # ---- layers 2, 3 ----
    for name, wflat in (("W2", w2), ("W3", w3)):
        b = split_blocks(wflat, HID_IRREPS, pre_irr)
        ws, wg = b[0][1], b[1][1]                    # [64,64], [64,256]
        add(name + "_0", np.concatenate([ws, wg[:, gperm]], axis=1))   # [64, 320]
        wl = {l: w for l, w in b[2:]}
        wa = np.zeros((128, 128), np.float32)
        wa[0:64, 0:64] = wl[2]
        wa[64:128, 64:128] = wl[1]
        add(name + "_A", wa)
        wb = np.zeros((128, 128), np.float32)
        for j, l in enumerate((6, 5, 4, 3)):
            wb[32 * j:32 * (j + 1), 32 * j:32 * (j + 1)] = wl[l]
        add(name + "_B", wb)

    # ---- layer 4 ----
    b4 = split_blocks(w4, HID_IRREPS, [(1, l) for l in range(7)])
    w4l = {l: w[:, 0] for l, w in b4}
    OUT_OFF = {l: l * l for l in range(7)}
    w40 = np.zeros((64, D_OUT), np.float32)
    w40[:, 0] = w4l[0]
    add("W4_0", w40)
    for m in range(5):
        wa = np.zeros((128, D_OUT), np.float32)
        wa[0:64, OUT_OFF[2] + m] = w4l[2]
        if m < 3:
            wa[64:128, OUT_OFF[1] + m] = w4l[1]
        add(f"W4_A{m}", wa)
    for m in range(13):
        wbm = np.zeros((128, D_OUT), np.float32)
        wbm[0:32, OUT_OFF[6] + m] = w4l[6]
        if m < 11:
            wbm[32:64, OUT_OFF[5] + m] = w4l[5]
        if m < 9:
            wbm[64:96, OUT_OFF[4] + m] = w4l[4]
        if m < 7:
            wbm[96:128, OUT_OFF[3] + m] = w4l[3]
        add(f"W4_B{m}", wbm)

    col = {}
    off = 0
    for name, a in segs:
        col[name] = off
        off += a.shape[1]
    wt = np.concatenate([a for _, a in segs], axis=1).astype(bf)
    return wt, col


def _pack_v(v_raw):
    """[B_FULL, 3840] fp32 -> [128, NSLOT, B_FULL] bf16 slot layout."""
    B = v_raw.shape[0]
    vt = np.zeros((128, NSLOT, B), np.float32)
    v0 = v_raw[:, IN_OFF[0]:IN_OFF[0] + 256]
    vt[:, S_L0 + 0, :] = v0[:, 0:128].T
    vt[:, S_L0 + 1, :] = v0[:, 128:256].T
    for l, base, nm in ((1, S_L1, 3), (2, S_L2, 5)):
        d = 2 * l + 1
        vb = v_raw[:, IN_OFF[l]:IN_OFF[l] + 128 * d].reshape(B, 128, d)
        for m in range(nm):
            vt[:, base + m, :] = vb[:, :, m].T
    for l, base, rows in ((6, S_B56, slice(0, 64)), (5, S_B56, slice(64, 128)),
                          (4, S_B34, slice(0, 64)), (3, S_B34, slice(64, 128))):
        d = 2 * l + 1
        vb = v_raw[:, IN_OFF[l]:IN_OFF[l] + 64 * d].reshape(B, 64, d)
        for m in range(d):
            vt[rows, base + m, :] = vb[:, :, m].T
    return vt.astype(bf)


def _split_excess_waits(nc, max_waits=1):
    """Walrus accepts only one sem-wait on some ops; hoist excess waits
    onto same-engine NoOps inserted before."""
    for f in nc.m.functions:
        for bb in f.blocks:
            newlist = []
            changed = False
            for ins in bb.instructions:
                si = ins.sync_info
                waits = list(si.on_wait) if (si and si.on_wait) else []
                if len(waits) > max_waits:
                    extras, keep = waits[:-max_waits], waits[-max_waits:]
                    for k in range(0, len(extras), max_waits):
                        nop = mybir.InstNoOp(
                            name=f"{ins.name}_waitnop{k}", ins=[], outs=[],
                            engine=ins.engine)
                        nop.sync_info = mybir.SyncInfo(
                            on_wait=extras[k:k + max_waits], on_update=[])
                        nc.register_instruction(nop)
                        newlist.append(nop)
                    ins.sync_info = mybir.SyncInfo(
                        on_wait=keep,
                        on_update=list(si.on_update) if si.on_update else [])
                    changed = True
                newlist.append(ins)
            if changed:
                bb.instructions[:] = newlist
    return nc


def _build_program(col, repeat=1):
    nc = bass.Bass("TRN2", target_bir_lowering=False, debug=False)
    NWC = max(col.values()) + D_OUT   # W4_B12 is last
    vt_d = nc.dram_tensor("vt", [128, NSLOT, BC], BF16, kind="ExternalInput").ap()
    wt_d = nc.dram_tensor("wt", [128, NWC], BF16, kind="ExternalInput").ap()
    out_d = nc.dram_tensor("out", [D_OUT, BC], FP32, kind="ExternalOutput").ap()

    Sig = mybir.ActivationFunctionType.Sigmoid
    Silu = mybir.ActivationFunctionType.Silu
    Mult = mybir.AluOpType.mult

    with tile.TileContext(nc) as tc:
        with ExitStack() as ctx:
            pool = ctx.enter_context(tc.tile_pool(name="p", bufs=1))
            pp = ctx.enter_context(tc.tile_pool(name="ps", bufs=1, space="PSUM"))

            wt = pool.tile([128, NWC], BF16, tag="wt")
            nc.sync.dma_start(out=wt, in_=wt_d)

            vs = pool.tile([128, NSLOT * BT], BF16, tag="vs")   # staged b-tile
            out49 = pool.tile([D_OUT, BC], FP32, tag="out49")
            h0a = pool.tile([64, BT], BF16, tag="h0a")
            h0b = pool.tile([64, BT], BF16, tag="h0b")
            h0c = pool.tile([64, BT], BF16, tag="h0c")
            hA = pool.tile([128, 5 * BT], BF16, tag="hA")
            hB = pool.tile([128, 13 * BT], BF16, tag="hB")
            gAB1 = pool.tile([128, 2, BT], BF16, tag="gAB1")
            gAB2 = pool.tile([128, 2, BT], BF16, tag="gAB2")
            gAB3 = pool.tile([128, 2, BT], BF16, tag="gAB3")

            z0 = pp.tile([128, 3, BT], FP32, tag="z0")
            zP1 = pp.tile([128, BT], FP32, tag="zP1")
            zP2 = pp.tile([128, BT], FP32, tag="zP2")
            zP3 = pp.tile([128, BT], FP32, tag="zP3")
            z4 = pp.tile([D_OUT, BT], FP32, tag="z4")

            def W(name, w):
                c = col[name]
                return wt[:, c:c + w]

            def W64(name, w):
                c = col[name]
                return wt[0:64, c:c + w]

            mm = nc.tensor.matmul

            # static slot views of the staged tile
            def vslot(s):
                return vs[:, s * BT:(s + 1) * BT]

            vsA1 = vs[:, S_L1 * BT:(S_L1 + 5) * BT]
            vsA2 = vs[:, S_L2 * BT:(S_L2 + 5) * BT]
            vsB56 = vs[:, S_B56 * BT:(S_B56 + 13) * BT]
            vsB34 = vs[:, S_B34 * BT:(S_B34 + 13) * BT]

            def emit_body(t):
                nc.sync.dma_start(out=vs, in_=vt_d[:, :, ds(t * BT, BT)])

                # ---- gate chain: z0 of layers 1..3 (independent of A/B parts) ----
                for k in range(2):
                    wk = ("W1_0a", "W1_0b")[k]
                    st, sp = (k == 0), (k == 1)
                    x = vslot(S_L0 + k)
                    mm(z0[0:64, 0, :], W(wk, 320)[:, 0:64], x, start=st, stop=sp)
                    mm(z0[:, 1, :], W(wk, 320)[:, 64:192], x, start=st, stop=sp)
                    mm(z0[:, 2, :], W(wk, 320)[:, 192:320], x, start=st, stop=sp)
                nc.scalar.activation(gAB1, z0[:, 1:3, :], Sig)
                nc.scalar.activation(h0a, z0[0:64, 0, :], Silu)
                for Wn, gg, hin, hout in (("W2", gAB2, h0a, h0b),
                                          ("W3", gAB3, h0b, h0c)):
                    w0 = W64(Wn + "_0", 320)
                    mm(z0[0:64, 0, :], w0[:, 0:64], hin, start=True, stop=True)
                    mm(z0[:, 1, :], w0[:, 64:192], hin, start=True, stop=True)
                    mm(z0[:, 2, :], w0[:, 192:320], hin, start=True, stop=True)
                    nc.scalar.activation(gg, z0[:, 1:3, :], Sig)
                    nc.scalar.activation(hout, z0[0:64, 0, :], Silu)

                # ---- A superloop: slot m through layers 1->2->3 ----
                with tc.For_i(0, 5, 1) as m:
                    mm(zP1[0:64, :], W("W1_l2", 64), vsA2[:, ds(m * BT, BT)],
                       start=True, stop=True, tile_position=(0, 0))
                    mm(zP1[64:128, :], W("W1_l1", 64), vsA1[:, ds(m * BT, BT)],
                       start=True, stop=True, tile_position=(0, 64))
                    nc.vector.tensor_tensor(out=hA[:, ds(m * BT, BT)], in0=zP1,
                                            in1=gAB1[:, 0, :], op=Mult)
                    mm(zP2, W("W2_A", 128), hA[:, ds(m * BT, BT)],
                       start=True, stop=True)
                    nc.vector.tensor_tensor(out=hA[:, ds(m * BT, BT)], in0=zP2,
                                            in1=gAB2[:, 0, :], op=Mult)
                    mm(zP3, W("W3_A", 128), hA[:, ds(m * BT, BT)],
                       start=True, stop=True)
                    nc.vector.tensor_tensor(out=hA[:, ds(m * BT, BT)], in0=zP3,
                                            in1=gAB3[:, 0, :], op=Mult)

                # ---- B superloop ----
                with tc.For_i(0, 13, 1) as m:
                    mm(zP1[0:64, :], W("W1_b56", 64), vsB56[:, ds(m * BT, BT)],
                       start=True, stop=True, tile_position=(0, 0))
                    mm(zP1[64:128, :], W("W1_b34", 64), vsB34[:, ds(m * BT, BT)],
                       start=True, stop=True, tile_position=(0, 64))
                    nc.vector.tensor_tensor(out=hB[:, ds(m * BT, BT)], in0=zP1,
                                            in1=gAB1[:, 1, :], op=Mult)
                    mm(zP2, W("W2_B", 128), hB[:, ds(m * BT, BT)],
                       start=True, stop=True)
                    nc.vector.tensor_tensor(out=hB[:, ds(m * BT, BT)], in0=zP2,
                                            in1=gAB2[:, 1, :], op=Mult)
                    mm(zP3, W("W3_B", 128), hB[:, ds(m * BT, BT)],
                       start=True, stop=True)
                    nc.vector.tensor_tensor(out=hB[:, ds(m * BT, BT)], in0=zP3,
                                            in1=gAB3[:, 1, :], op=Mult)

                # ---------------- layer 4 ----------------
                mm(z4, W64("W4_0", D_OUT), h0c, start=True, stop=False)
                for m in range(5):
                    mm(z4, W(f"W4_A{m}", D_OUT), hA[:, m * BT:(m + 1) * BT],
                       start=False, stop=False)
                for m in range(13):
                    mm(z4, W(f"W4_B{m}", D_OUT), hB[:, m * BT:(m + 1) * BT],
                       start=False, stop=(m == 12))
                nc.vector.tensor_copy(out49[:, ts(t, BT)], z4)

            if repeat == 1:
                with tc.For_i(0, NT, 1) as t:
                    emit_body(t)
            else:
                with tc.For_i(0, repeat, 1) as r:
                    with tc.For_i(0, NT, 1) as t:
                        emit_body(t)

            nc.sync.dma_start(out=out_d, in_=out49)

    _split_excess_waits(nc)
    return nc


def _get_nc(col):
    key = ("nc", REPEAT)
    if key not in _BUILD:
        _BUILD[key] = _build_program(col, repeat=REPEAT)
    return _BUILD[key]


def kernel(v_raw, w1, w2, w3, w4):
    wt, col = _pack_weights(np.asarray(w1, np.float32), np.asarray(w2, np.float32),
                            np.asarray(w3, np.float32), np.asarray(w4, np.float32))
    nc = _get_nc(col)
    vt = _pack_v(np.asarray(v_raw, np.float32))     # [128, NSLOT, B_FULL] bf16
    in_maps = []
    for c in range(NCORES):
        vc = np.ascontiguousarray(vt[:, :, c * BC:(c + 1) * BC])
        in_maps.append({"vt": vc, "wt": wt})
    res = bass_utils.run_bass_kernel_spmd(nc, in_maps, core_ids=list(range(NCORES)))
    global LAST_RESULT
    LAST_RESULT = res
    full = np.empty((B_FULL, D_OUT), np.float32)
    for c in range(NCORES):
        full[c * BC:(c + 1) * BC, :] = res.results[c]["out"].T
    return full.reshape(B_FULL, D_OUT, 1)
